# revision 26
# baseline (speedup 1.0000x reference)
"""GCN+JumpingKnowledge distributed Trainium2 kernel (8 NeuronCores), v2.

Strategy: shard destination nodes across 8 cores (6250 each). Per layer:
  - z^T = W @ act^T on TensorE; scale columns by dinv[node] on DVE
    (so gathered rows carry dinv[src]); transpose + store rows to HBM
  - TWO AllGathers (first/second half of each core's shard) so the
    first half's edge gathers overlap the second collective
  - dma_gather source rows for this core's edges (sorted by dst tile,
    split by which half-table holds the src; idx=-1 pads skipped via
    runtime num_idxs_reg; calls round-robin 4 SWDGE queues for 3x
    drain throughput)
  - segment-sum via TensorE: psum[feat,dst] += G_blk^T @ S_blk with
    S = one-hot(dstoff) built in one DVE op per block
  - conv = (psum + z_scaled_local) * dinv[dst] + b, with the self-loop
    term folded in via the local scaled activations (no gather slots
    for self loops)
  - BN stats per-tile accumulators + 1KB AllReduce, fused
    scale/shift/ReLU on ACT; JK max fused; projection on device.
"""

import os
import sys

import numpy as np

sys.path.insert(0, "/opt/trn_rl_repo")

N = 50000
E = 800000
F = 128
OUTF = 64
N_CORES = 8
SHARD = N // N_CORES  # 6250
HSHARD = SHARD // 2  # 3125
TABLE = N_CORES * HSHARD  # 25000 rows per half-table
TILE = 128
NTILE = (SHARD + TILE - 1) // TILE  # 49
LAST_W = SHARD - (NTILE - 1) * TILE  # 106
GRP = 4  # tiles per gather group
BN_EPS = 1e-5
ZCHUNK = 512
MAX_CALL = 1024
RTCOUNT = os.environ.get("KGNN_RTCOUNT", "1") == "1"


def _preprocess(edge_index):
    """Host-side edge routing. Returns (structure, per_core_arrays)."""
    src = np.asarray(edge_index[0], dtype=np.int64)
    dst = np.asarray(edge_index[1], dtype=np.int64)

    deg = np.bincount(dst, minlength=N).astype(np.float64) + 1.0
    dinv = (1.0 / np.sqrt(deg)).astype(np.float64)

    core = dst // SHARD
    loc_d = dst % SHARD
    tile_id = loc_d // TILE
    dstoff = (loc_d % TILE).astype(np.float32)
    loc_s = src % SHARD
    half = (loc_s >= HSHARD).astype(np.int64)
    tblidx = ((src // SHARD) * HSHARD + loc_s - half * HSHARD).astype(np.int64)

    # per (core, tile, half) counts
    key = (core * NTILE + tile_id) * 2 + half
    counts = np.bincount(key, minlength=N_CORES * NTILE * 2).reshape(
        N_CORES, NTILE, 2
    )
    maxcnt = counts.max(axis=0)  # [NTILE, 2]
    pad_blocks = (maxcnt + TILE - 1) // TILE

    # slot layout: groups of GRP tiles; per group all A slots then all B.
    groups = []
    slot_start = np.zeros((NTILE, 2), dtype=np.int64)
    cursor = 0
    ncalls = 0
    for g0 in range(0, NTILE, GRP):
        tiles = list(range(g0, min(g0 + GRP, NTILE)))
        ginfo = {"tiles": tiles}
        for h, nm in ((0, "lo"), (1, "hi")):
            tb = []
            calls = []
            # calls never span buckets: the ucode requires negative (skip)
            # indices to be trailing within each call window.
            for t in tiles:
                slot_start[t, h] = cursor
                nbk = int(pad_blocks[t, h])
                tb.append((cursor, nbk))
                end = cursor + nbk * TILE
                o = cursor
                while o < end:
                    n = min(MAX_CALL, end - o)
                    calls.append((o, n, ncalls))
                    ncalls += 1
                    o += n
                cursor = end
            ginfo[nm] = {"tile_blocks": tb, "calls": calls}
        groups.append(ginfo)
    total_slots = cursor
    total_blocks = total_slots // TILE

    # uniform keep mask: slot kept iff rank < maxcnt of its bucket. Kept
    # counts are identical across cores, so num_idxs_reg can be a compile
    # time immediate; only the block-rounding padding is skipped.
    keep = np.zeros(total_slots, dtype=bool)
    for t in range(NTILE):
        for h in (0, 1):
            keep[slot_start[t, h] : slot_start[t, h] + maxcnt[t, h]] = True
    ucounts = np.zeros(ncalls, dtype=np.int64)
    for g in groups:
        for nm in ("lo", "hi"):
            for (cs0, cns, ci) in g[nm]["calls"]:
                u = int(keep[cs0 : cs0 + cns].sum())
                if u == 0:
                    keep[cs0] = True
                    u = 1
                ucounts[ci] = u

    per_core = []
    for c in range(N_CORES):
        m = core == c
        e_t = tile_id[m]
        e_h = half[m]
        e_idx = tblidx[m]
        e_do = dstoff[m]
        order = np.lexsort((e_idx, e_h, e_t))
        e_t, e_h = e_t[order], e_h[order]
        e_idx, e_do = e_idx[order], e_do[order]
        k = e_t * 2 + e_h
        cnt_c = np.bincount(k, minlength=NTILE * 2)
        grp_starts = np.concatenate([[0], np.cumsum(cnt_c)[:-1]])
        rank = np.arange(len(k)) - grp_starts[k]
        slots = slot_start[e_t, e_h] + rank

        if RTCOUNT:
            idx_vals = np.where(keep, 0, -1).astype(np.int16)
        else:
            idx_vals = np.zeros(total_slots, dtype=np.int16)
        doff = np.full(total_slots, 999.0, dtype=np.float32)
        idx_vals[slots] = e_idx.astype(np.int16)
        doff[slots] = e_do

        # per-call counts (uniform across cores; kernel uses immediates)
        call_counts = np.asarray(ucounts, dtype=np.int32)

        # idx wrapped layout: slot i -> partition i%16 (replicated x8)
        idx_arr = np.zeros((128, total_slots // 16), dtype=np.int16)
        v16 = idx_vals.reshape(-1, 16).T
        for g in range(8):
            idx_arr[16 * g : 16 * g + 16] = v16
        doff_arr = np.ascontiguousarray(doff.reshape(-1, 128).T)

        dinv_core = dinv[c * SHARD : (c + 1) * SHARD].astype(np.float32)
        dinvrep = np.ascontiguousarray(
            np.broadcast_to(dinv_core[None, :], (128, SHARD))
        )
        per_core.append(
            {
                "idx": idx_arr,
                "dstoff": doff_arr,
                "counts": call_counts.reshape(1, -1),
                "dinvrep": dinvrep,
            }
        )

    structure = {
        "groups": groups,
        "total_slots": total_slots,
        "total_blocks": total_blocks,
        "ncalls": ncalls,
        "ucounts": [int(u) for u in ucounts],
    }
    return structure, per_core


def _build(structure):
    import concourse.bacc as bacc
    import concourse.tile as tile
    from concourse import mybir

    fp32 = mybir.dt.float32
    fp16 = mybir.dt.float16
    i16 = mybir.dt.int16
    i32 = mybir.dt.int32
    AF = mybir.ActivationFunctionType
    OP = mybir.AluOpType

    groups = structure["groups"]
    total_slots = structure["total_slots"]
    total_blocks = structure["total_blocks"]
    ncalls = structure["ncalls"]

    nc = bacc.Bacc(
        "TRN2", target_bir_lowering=False, num_devices=N_CORES, num_swdge_queues=4
    )

    # ---- I/O ----
    xT_in = nc.declare_dram_parameter("xT", [F, SHARD], fp16, isOutput=False)
    idx_in = nc.declare_dram_parameter("idx", [128, total_slots // 16], i16, isOutput=False)
    doff_in = nc.declare_dram_parameter("dstoff", [128, total_blocks], fp32, isOutput=False)
    counts_in = nc.declare_dram_parameter("counts", [1, ncalls], i32, isOutput=False)
    dinvrep_in = nc.declare_dram_parameter("dinvrep", [128, SHARD], fp32, isOutput=False)
    w_in = [
        nc.declare_dram_parameter(f"W{i}", [F, F], fp16, isOutput=False)
        for i in (1, 2, 3)
    ]
    wp_in = nc.declare_dram_parameter("Wp", [F, OUTF], fp16, isOutput=False)
    b_in = [
        nc.declare_dram_parameter(f"b{i}", [F, 1], fp32, isOutput=False)
        for i in (1, 2, 3)
    ]
    bp_in = nc.declare_dram_parameter("bp", [OUTF, 1], fp32, isOutput=False)
    g_in = [
        nc.declare_dram_parameter(f"g{i}", [F, 1], fp32, isOutput=False) for i in (1, 2)
    ]
    be_in = [
        nc.declare_dram_parameter(f"be{i}", [F, 1], fp32, isOutput=False)
        for i in (1, 2)
    ]
    out_ext = nc.declare_dram_parameter("outT", [OUTF, SHARD], fp32, isOutput=True)

    with tile.TileContext(nc) as tc:
        from contextlib import ExitStack

        with ExitStack() as ctx:
            dram = ctx.enter_context(tc.tile_pool(name="dram", bufs=1, space="DRAM"))
            singles = ctx.enter_context(tc.tile_pool(name="singles", bufs=1))
            glo_p = ctx.enter_context(tc.tile_pool(name="glo", bufs=12))
            ghi_p = ctx.enter_context(tc.tile_pool(name="ghi", bufs=12))
            s_p = ctx.enter_context(tc.tile_pool(name="spool", bufs=4))
            a_p = ctx.enter_context(tc.tile_pool(name="apool", bufs=3))
            conv_ps = ctx.enter_context(tc.tile_pool(name="convps", bufs=3, space="PSUM"))
            z_ps = ctx.enter_context(tc.tile_pool(name="zps", bufs=2, space="PSUM"))
            t_ps = ctx.enter_context(tc.tile_pool(name="tps", bufs=2, space="PSUM"))
            rstage = ctx.enter_context(tc.tile_pool(name="rstage", bufs=3))
            small = ctx.enter_context(tc.tile_pool(name="small", bufs=2))

            # DRAM internals
            z_shards = [dram.tile([SHARD, F], fp16, name=f"z_shard{i}") for i in range(3)]
            z_fullA = [
                dram.tile([TABLE, F], fp16, name=f"z_fullA{i}") for i in range(3)
            ]
            z_fullB = [
                dram.tile([TABLE, F], fp16, name=f"z_fullB{i}") for i in range(3)
            ]
            stats_locs = [dram.tile([F, 2], fp32, name=f"stats_loc{i}") for i in range(2)]
            stats_globs = [
                dram.tile([F, 2], fp32, addr_space="Shared", name=f"stats_glob{i}")
                for i in range(2)
            ]

            # ---- load constants ----
            idx_sb = singles.tile([128, total_slots // 16], i16)
            nc.sync.dma_start(out=idx_sb[:], in_=idx_in[:])
            doff_sb = singles.tile([128, total_blocks], fp32)
            nc.sync.dma_start(out=doff_sb[:], in_=doff_in[:])
            counts_sb = singles.tile([1, ncalls], i32)
            nc.sync.dma_start(out=counts_sb[:], in_=counts_in[:])
            doff16_sb = singles.tile([128, total_blocks], fp16)
            nc.vector.tensor_copy(doff16_sb[:], doff_sb[:])
            dinvrep_sb = singles.tile([128, SHARD], fp32)
            nc.sync.dma_start(out=dinvrep_sb[:], in_=dinvrep_in[:])
            w_sb = []
            for i in range(3):
                w = singles.tile([F, F], fp16, name=f"w{i}")
                nc.sync.dma_start(out=w[:], in_=w_in[i][:])
                w_sb.append(w)
            wp_sb = singles.tile([F, OUTF], fp16)
            nc.sync.dma_start(out=wp_sb[:], in_=wp_in[:])
            b_sb = []
            for i in range(3):
                b = singles.tile([F, 1], fp32, name=f"b{i}")
                nc.sync.dma_start(out=b[:], in_=b_in[i][:])
                b_sb.append(b)
            bp_sb = singles.tile([OUTF, 1], fp32)
            nc.sync.dma_start(out=bp_sb[:], in_=bp_in[:])
            g_sb, be_sb = [], []
            for i in range(2):
                g = singles.tile([F, 1], fp32, name=f"g{i}")
                nc.sync.dma_start(out=g[:], in_=g_in[i][:])
                g_sb.append(g)
                be = singles.tile([F, 1], fp32, name=f"be{i}")
                nc.sync.dma_start(out=be[:], in_=be_in[i][:])
                be_sb.append(be)

            iota_sb = singles.tile([128, 128], fp16)
            nc.gpsimd.iota(
                iota_sb[:],
                pattern=[[1, 128]],
                base=0,
                channel_multiplier=0,
                allow_small_or_imprecise_dtypes=True,
            )
            ident_sb = singles.tile([128, 128], fp16)
            identi32 = singles.tile([128, 128], mybir.dt.int32)
            nc.gpsimd.iota(
                identi32[:], pattern=[[1, 128]], base=0, channel_multiplier=-1
            )
            nc.vector.tensor_scalar(
                out=ident_sb[:],
                in0=identi32[:],
                scalar1=0.0,
                scalar2=None,
                op0=OP.is_equal,
            )

            # persistent activations
            actA = singles.tile([F, SHARD], fp16)  # layer input act^T
            nc.sync.dma_start(out=actA[:], in_=xT_in[:])
            actB = singles.tile([F, SHARD], fp16)
            z_scaledT = singles.tile([F, SHARD], fp16)  # z * dinv[node]
            zl_dd = singles.tile([F, SHARD], fp32)  # z_scaled * dinv[dst]
            conv_sb = singles.tile([F, SHARD], fp32)
            sumcols = singles.tile([F, NTILE], fp32)
            sqcols = singles.tile([F, NTILE], fp32)
            sq_scratch = singles.tile([F, TILE], fp32)

            zchunks = []
            o = 0
            while o < SHARD:
                w = min(ZCHUNK, SHARD - o)
                zchunks.append((o, w))
                o += w

            def produce_z(act_src, w_idx):
                """z_scaledT = (W @ act^T) * dinv; rows to z_shard; 2 AllGathers.
                AG of the first half is issued as soon as its rows are stored."""
                z_shard = z_shards[w_idx]
                agA_done = False
                for (o, w) in zchunks:
                    zp = z_ps.tile([F, ZCHUNK], fp32, tag="zps")
                    nc.tensor.matmul(
                        zp[:, :w], lhsT=w_sb[w_idx][:], rhs=act_src[:, o : o + w],
                        start=True, stop=True,
                    )
                    nc.vector.tensor_tensor(
                        z_scaledT[:, o : o + w],
                        zp[:, :w],
                        dinvrep_sb[:, o : o + w],
                        op=OP.mult,
                    )
                    k = 0
                    while k < w:
                        wk = min(128, w - k)
                        tp = t_ps.tile([128, 128], fp16, tag="tps")
                        nc.tensor.transpose(
                            tp[:wk, :], z_scaledT[:, o + k : o + k + wk], ident_sb[:]
                        )
                        zr = rstage.tile([128, F], fp16, tag="zrow")
                        nc.vector.tensor_copy(zr[:wk, :], tp[:wk, :])
                        nc.sync.dma_start(
                            out=z_shard[o + k : o + k + wk, :], in_=zr[:wk, :]
                        )
                        k += wk
                    if not agA_done and o + w >= HSHARD:
                        agA_done = True
                        nc.gpsimd.collective_compute(
                            "AllGather",
                            mybir.AluOpType.bypass,
                            replica_groups=[list(range(N_CORES))],
                            ins=[z_shard[0:HSHARD, :].opt()],
                            outs=[z_fullA[w_idx][:].opt()],
                        )
                nc.gpsimd.collective_compute(
                    "AllGather",
                    mybir.AluOpType.bypass,
                    replica_groups=[list(range(N_CORES))],
                    ins=[z_shard[HSHARD:SHARD, :].opt()],
                    outs=[z_fullB[w_idx][:].opt()],
                )
                # zl_dd = z_scaledT * dinv[dst]  (self-loop term, fp32)
                nc.vector.tensor_tensor(
                    zl_dd[:], z_scaledT[:], dinvrep_sb[:], op=OP.mult
                )

            qctr = [0]
            ucounts = structure["ucounts"]

            def emit_gathers(run, z_full, pool, nm, blockmap):
                src_ap = z_full[:, :]
                for (cs0, cns, ci) in run["calls"]:
                    nblk = cns // TILE
                    gbuf = pool.tile([128, nblk, F], fp16, tag=f"g{nm}")
                    nreg = ucounts[ci] if RTCOUNT else cns
                    nc.gpsimd.dma_gather(
                        gbuf[:],
                        src_ap,
                        idx_sb[:, cs0 // 16 : (cs0 + cns) // 16],
                        cns,
                        nreg,
                        F,
                        queue_num=qctr[0] % 4,
                    )
                    qctr[0] += 1
                    for j in range(nblk):
                        blockmap[cs0 // TILE + j] = (gbuf, j)

            def emit_seg_matmuls(cps, gb0, nb, blockmap, bi, total):
                """S one-hot builds (4-block batched) + psum-accumulated matmuls."""
                for c0 in range(0, nb, 4):
                    w = min(4, nb - c0)
                    s4 = s_p.tile([128, 4, 128], fp16, tag="s")
                    nc.vector.tensor_tensor(
                        s4[:, :w, :],
                        iota_sb[:].unsqueeze(1).broadcast_to([128, w, 128]),
                        doff16_sb[:, gb0 + c0 : gb0 + c0 + w]
                        .unsqueeze(2)
                        .broadcast_to([128, w, 128]),
                        op=OP.is_equal,
                    )
                    for j in range(w):
                        gbuf, lb = blockmap[gb0 + c0 + j]
                        nc.tensor.matmul(
                            cps[:],
                            lhsT=gbuf[:, lb, :],
                            rhs=s4[:, j, :],
                            start=(bi == 0),
                            stop=(bi == total - 1),
                        )
                        bi += 1
                return bi

            def emit_lo_tiles(ginfo, blockmap):
                """lo-half segment sums -> conv_sb (as accumulator)."""
                for ti, t in enumerate(ginfo["tiles"]):
                    s0, nb = ginfo["lo"]["tile_blocks"][ti]
                    tw = LAST_W if t == NTILE - 1 else TILE
                    o = t * TILE
                    if not nb:
                        nc.vector.memset(conv_sb[:, o : o + tw], 0.0)
                        continue
                    cps = conv_ps.tile([F, TILE], fp32, tag="convps")
                    emit_seg_matmuls(cps, s0 // TILE, nb, blockmap, 0, nb)
                    nc.vector.tensor_copy(conv_sb[:, o : o + tw], cps[:, :tw])

            def emit_hi_tiles(lyr, ginfo, blockmap, need_stats):
                """hi-half segment sums + combine: conv = (acc+hi)*dinvd + b + zl."""
                for ti, t in enumerate(ginfo["tiles"]):
                    s0, nb = ginfo["hi"]["tile_blocks"][ti]
                    tw = LAST_W if t == NTILE - 1 else TILE
                    o = t * TILE
                    a_t = a_p.tile([F, TILE], fp32, tag="a")
                    if nb:
                        cps = conv_ps.tile([F, TILE], fp32, tag="convps")
                        emit_seg_matmuls(cps, s0 // TILE, nb, blockmap, 0, nb)
                        nc.vector.tensor_tensor(
                            a_t[:, :tw], cps[:, :tw], conv_sb[:, o : o + tw],
                            op=OP.add,
                        )
                    else:
                        nc.vector.tensor_copy(a_t[:, :tw], conv_sb[:, o : o + tw])
                    nc.vector.tensor_tensor(
                        a_t[:, :tw], a_t[:, :tw], dinvrep_sb[:, o : o + tw],
                        op=OP.mult,
                    )
                    nc.vector.scalar_tensor_tensor(
                        out=conv_sb[:, o : o + tw],
                        in0=a_t[:, :tw],
                        scalar=b_sb[lyr][:],
                        in1=zl_dd[:, o : o + tw],
                        op0=OP.add,
                        op1=OP.add,
                        accum_out=sumcols[:, t : t + 1] if need_stats else None,
                    )
                    if need_stats:
                        nc.scalar.activation(
                            out=sq_scratch[:, :tw],
                            in_=conv_sb[:, o : o + tw],
                            func=AF.Square,
                            accum_out=sqcols[:, t : t + 1],
                        )

            def conv_layer(lyr):
                need_stats = lyr < 2
                blockmap = {}
                LA = 1
                ng = len(groups)
                # phase 1: all lo gathers + lo tile sums (AG_B hides under this)
                for gi in range(ng + LA):
                    if gi < ng:
                        emit_gathers(
                            groups[gi]["lo"], z_fullA[lyr], glo_p, "lo", blockmap
                        )
                    if gi >= LA:
                        emit_lo_tiles(groups[gi - LA], blockmap)
                # phase 2: all hi gathers + combine
                for gi in range(ng + LA):
                    if gi < ng:
                        emit_gathers(
                            groups[gi]["hi"], z_fullB[lyr], ghi_p, "hi", blockmap
                        )
                    if gi >= LA:
                        emit_hi_tiles(lyr, groups[gi - LA], blockmap, need_stats)

            def bn_relu(lyr, act_out):
                ssum = small.tile([F, 1], fp32, tag="ssum")
                nc.vector.tensor_reduce(
                    ssum[:], sumcols[:], axis=mybir.AxisListType.X, op=OP.add
                )
                ssq = small.tile([F, 1], fp32, tag="ssq")
                nc.vector.tensor_reduce(
                    ssq[:], sqcols[:], axis=mybir.AxisListType.X, op=OP.add
                )
                st = small.tile([F, 2], fp32, tag="stats")
                nc.vector.tensor_copy(st[:, 0:1], ssum[:])
                nc.vector.tensor_copy(st[:, 1:2], ssq[:])
                nc.sync.dma_start(out=stats_locs[lyr][:], in_=st[:])
                nc.gpsimd.collective_compute(
                    "AllReduce",
                    OP.add,
                    replica_groups=[list(range(N_CORES))],
                    ins=[stats_locs[lyr][:].opt()],
                    outs=[stats_globs[lyr][:].opt()],
                )
                stg = small.tile([F, 2], fp32, tag="statsg")
                nc.sync.dma_start(out=stg[:], in_=stats_globs[lyr][:])
                mean = small.tile([F, 1], fp32, tag="mean")
                nc.vector.tensor_scalar_mul(mean[:], stg[:, 0:1], 1.0 / N)
                ex2 = small.tile([F, 1], fp32, tag="ex2")
                nc.vector.tensor_scalar_mul(ex2[:], stg[:, 1:2], 1.0 / N)
                var = small.tile([F, 1], fp32, tag="var")
                nc.vector.tensor_tensor(var[:], mean[:], mean[:], op=OP.mult)
                nc.vector.tensor_sub(var[:], ex2[:], var[:])
                nc.vector.tensor_scalar_add(var[:], var[:], BN_EPS)
                std = small.tile([F, 1], fp32, tag="std")
                nc.scalar.sqrt(std[:], var[:])
                rstd = small.tile([F, 1], fp32, tag="rstd")
                nc.vector.reciprocal(rstd[:], std[:])
                scale = small.tile([F, 1], fp32, tag="scale")
                nc.vector.tensor_mul(scale[:], rstd[:], g_sb[lyr][:])
                shift = small.tile([F, 1], fp32, tag="shift")
                nc.vector.tensor_mul(shift[:], mean[:], scale[:])
                nc.vector.tensor_sub(shift[:], be_sb[lyr][:], shift[:])
                nc.scalar.activation(
                    out=act_out[:],
                    in_=conv_sb[:],
                    func=AF.Relu,
                    bias=shift[:],
                    scale=scale[:],
                )

            PHASE = int(os.environ.get("KGNN_PHASE", "9"))
            nc.vector.memset(actB[:], 0.0)
            nc.vector.memset(conv_sb[:], 0.0)
            if RTCOUNT:
                # first use of each gather buffer must be finite: skipped
                # (idx=-1) slots keep stale SBUF contents, and NaN*0 = NaN
                # in the segment matmul.
                for _ in range(12):
                    t = glo_p.tile([128, MAX_CALL // TILE, F], fp16, tag="glo")
                    nc.vector.memset(t[:], 0.0)
                    t = ghi_p.tile([128, MAX_CALL // TILE, F], fp16, tag="ghi")
                    nc.vector.memset(t[:], 0.0)
            # ---- layer 1 ----
            if PHASE >= 1:
                produce_z(actA, 0)
            if PHASE >= 2:
                conv_layer(0)
            if PHASE >= 3:
                bn_relu(0, actB)
            if PHASE >= 4:
                # ---- layer 2 ----
                produce_z(actB, 1)
                conv_layer(1)
                bn_relu(1, actA)
                nc.vector.tensor_max(actB[:], actB[:], actA[:])
            if PHASE >= 5:
                # ---- layer 3 ----
                produce_z(actA, 2)
                conv_layer(2)
                # conv3 -> fp16, jk = max(jk12, conv3)
                nc.scalar.copy(actA[:], conv_sb[:])
                nc.vector.tensor_max(actB[:], actB[:], actA[:])
            # ---- projection ----
            for (o, w) in zchunks:
                pp = z_ps.tile([F, ZCHUNK], fp32, tag="zps")
                nc.tensor.matmul(
                    pp[:OUTF, :w], lhsT=wp_sb[:], rhs=actB[:, o : o + w],
                    start=True, stop=True,
                )
                po = rstage.tile([OUTF, ZCHUNK], fp32, tag="pout")
                nc.scalar.activation(
                    out=po[:, :w], in_=pp[:OUTF, :w], func=AF.Identity,
                    bias=bp_sb[:], scale=1.0,
                )
                nc.sync.dma_start(out=out_ext[:, o : o + w], in_=po[:, :w])

    nc.compile()
    return nc


_CACHE = {}
_LAST_RES = None


def kernel(**inputs):
    from concourse.bass_utils import run_bass_kernel_spmd

    x = np.asarray(inputs["x"], dtype=np.float32)
    edge_index = np.asarray(inputs["edge_index"])

    ck = hash(edge_index.tobytes())
    if ck not in _CACHE:
        structure, per_core = _preprocess(edge_index)
        nc = _build(structure)
        _CACHE[ck] = (structure, per_core, nc)
    structure, per_core, nc = _CACHE[ck]

    in_maps = []
    for c in range(N_CORES):
        xc = x[c * SHARD : (c + 1) * SHARD].astype(np.float16)
        m = {
            "xT": np.ascontiguousarray(xc.T),
            "idx": per_core[c]["idx"],
            "dstoff": per_core[c]["dstoff"],
            "counts": per_core[c]["counts"],
            "dinvrep": per_core[c]["dinvrep"],
            "W1": np.asarray(inputs["W1"], np.float16),
            "W2": np.asarray(inputs["W2"], np.float16),
            "W3": np.asarray(inputs["W3"], np.float16),
            "Wp": np.asarray(inputs["Wp"], np.float16),
            "b1": np.asarray(inputs["b1"], np.float32).reshape(F, 1),
            "b2": np.asarray(inputs["b2"], np.float32).reshape(F, 1),
            "b3": np.asarray(inputs["b3"], np.float32).reshape(F, 1),
            "bp": np.asarray(inputs["bp"], np.float32).reshape(OUTF, 1),
            "g1": np.asarray(inputs["g1"], np.float32).reshape(F, 1),
            "g2": np.asarray(inputs["g2"], np.float32).reshape(F, 1),
            "be1": np.asarray(inputs["be1"], np.float32).reshape(F, 1),
            "be2": np.asarray(inputs["be2"], np.float32).reshape(F, 1),
        }
        in_maps.append(m)

    trace = os.environ.get("KGNN_TRACE", "0") == "1"
    res = run_bass_kernel_spmd(
        nc,
        in_maps,
        core_ids=list(range(N_CORES)),
        trace=trace,
        trace_cores=list(range(N_CORES)) if trace else None,
    )
    global _LAST_RES
    _LAST_RES = res
    out = np.empty((N, OUTF), dtype=np.float32)
    for c in range(N_CORES):
        out[c * SHARD : (c + 1) * SHARD] = res.results[c]["outT"].T
    return out
